# revision 1
# baseline (speedup 1.0000x reference)
"""Causal self-attention (B=4, T=2048, C=1024, H=16, D=64) on 8 trn2 NeuronCores.

Sharding: tensor-parallel over heads. Each core owns 2 heads:
  - computes Q^T/K^T/V for its heads from the (host-pretransposed) full x^T,
  - causal attention (transposed-S flash-style, softmax denominator via an
    augmented ones-column on V),
  - partial output projection with its 128 rows of W_proj.
Host sums the 8 partial projections and adds (b_v @ W_proj + b_proj).

The per-core program is identical (SPMD); only the weight-slice inputs differ.
"""

import os

import numpy as np
import ml_dtypes

import concourse.bass as bass
import concourse.bacc as bacc
import concourse.mybir as mybir
import concourse.tile as tile

B, T, C, H, D = 4, 2048, 1024, 16, 64
NCORES = 8
HPC = H // NCORES  # heads per core = 2
P = 128
NB = T // P  # 16 blocks of 128 per sequence
CK = C // P  # 8 contraction chunks for the projections

F32 = mybir.dt.float32
F32R = mybir.dt.float32r
BF16 = mybir.dt.bfloat16
ADD = mybir.AluOpType.add
MULT = mybir.AluOpType.mult
EXP = mybir.ActivationFunctionType.Exp


def _segments():
    """(c, jlo, jhi): S^T strip for key-chunk c covering q-blocks [jlo, jhi).
    Width capped at 8 blocks (1024 cols = 2 PSUM banks)."""
    segs = []
    for c in range(NB):
        segs.append((c, c, min(c + 8, NB)))
    for c in range(8):
        if c + 8 < NB:
            segs.append((c, c + 8, NB))
    return segs


SEGS = _segments()


def _at_offsets():
    off = {}
    cur = 0
    for (c, jlo, jhi) in SEGS:
        for j in range(jlo, jhi):
            off[(c, j)] = cur + (j - jlo) * P
        cur += (jhi - jlo) * P
    return off, cur


AT_OFF, AT_W = _at_offsets()  # AT_W = 136*128 = 17408


def attention_body(tc, outs, ins):
    """Tile kernel body. outs/ins are dicts of bass.APs (DRAM)."""
    LEVEL = int(os.environ.get("KLEVEL", "7"))
    nc = tc.nc
    xt = ins["xt"]  # [C, B*T] bf16 (x transposed, col = b*T + t)
    wq = ins["wq"]  # [C, 128] bf16
    wk = ins["wk"]  # [C, 128] bf16
    wv = ins["wv"]  # [C, 128] bf16
    wp = ins["wp"]          # [128, C] f32
    bq = ins["bq"]          # [128, 1] f32
    bk = ins["bk"]          # [128, 1] f32
    maskt = ins["maskt"]    # [128, 128] bf16: 1 if k<=q else 0
    ident = ins["ident"]    # [128, 128] bf16 identity
    out = outs["out"]       # [B*T, C] f32 partial projection output

    with (
        tc.tile_pool(name="consts", bufs=1) as consts,
        tc.tile_pool(name="xtp", bufs=3) as xtp,
        tc.tile_pool(name="qkp", bufs=3) as qkp,
        tc.tile_pool(name="vp", bufs=3) as vp,
        tc.tile_pool(name="atp", bufs=2) as atp,
        tc.tile_pool(name="smallp", bufs=4) as smallp,
        tc.tile_pool(name="outp", bufs=4) as outp,
        tc.tile_pool(name="pp", bufs=4, space="PSUM") as pp,
    ):
        # ---- constants ----
        wq_sb = consts.tile([P, CK, P], BF16, name="wq_sb")
        nc.sync.dma_start(wq_sb, wq.rearrange("(o p) m -> p o m", p=P))
        wk_sb = consts.tile([P, CK, P], BF16, name="wk_sb")
        nc.sync.dma_start(wk_sb, wk.rearrange("(o p) m -> p o m", p=P))
        wv_sb = consts.tile([P, CK, P], BF16, name="wv_sb")
        nc.sync.dma_start(wv_sb, wv.rearrange("(o p) m -> p o m", p=P))
        wp_bf = consts.tile([P, C], BF16, name="wp_bf")
        nc.sync.dma_start(wp_bf, wp)
        bq_sb = consts.tile([P, 1], F32, name="bq_sb")
        nc.gpsimd.dma_start(bq_sb, bq)
        bk_sb = consts.tile([P, 1], F32, name="bk_sb")
        nc.gpsimd.dma_start(bk_sb, bk)
        mask_sb = consts.tile([P, P], BF16, name="mask_sb")
        nc.gpsimd.dma_start(mask_sb, maskt)
        id_sb = consts.tile([P, P], BF16, name="id_sb")
        nc.gpsimd.dma_start(id_sb, ident)

        for b in range(B):
            # ======== QKV phase: Q^T, K^T (f32), natural V (bf16, +ones col) ====
            qt = qkp.tile([P, T], BF16, tag="qt", name=f"qt_{b}")
            kt = qkp.tile([P, T], BF16, tag="kt", name=f"kt_{b}")
            vaug = vp.tile([P, NB, HPC, D + 1], BF16, tag="vaug", name=f"vaug_{b}")
            if LEVEL >= 4:
                nc.gpsimd.memset(vaug[:, :, :, D:], 1.0)  # softmax-denominator column

            for q4 in range(4):  # quarters of T (512 cols each)
                lo = q4 * 512
                xq = xtp.tile([P, CK, 512], BF16, tag="xq", name=f"xq_{b}_{q4}")
                nc.sync.dma_start(
                    xq, xt[:, b * T + lo : b * T + lo + 512].rearrange("(o p) t -> p o t", p=P)
                )
                if LEVEL < 2:
                    continue
                # Q^T quarter
                ps_q = pp.tile([P, 512], F32, tag="mm", name=f"psq_{b}_{q4}")
                for cc in range(CK):
                    nc.tensor.matmul(
                        ps_q, lhsT=wq_sb[:, cc], rhs=xq[:, cc],
                        start=(cc == 0), stop=(cc == CK - 1),
                    )
                # qt = psum * (1/sqrt(D)) + bq_prescaled
                nc.scalar.activation(
                    qt[:, lo : lo + 512], ps_q, mybir.ActivationFunctionType.Identity,
                    bias=bq_sb, scale=0.125,
                )
                if LEVEL < 3:
                    continue
                # K^T quarter
                ps_k = pp.tile([P, 512], F32, tag="mm", name=f"psk_{b}_{q4}")
                for cc in range(CK):
                    nc.tensor.matmul(
                        ps_k, lhsT=wk_sb[:, cc], rhs=xq[:, cc],
                        start=(cc == 0), stop=(cc == CK - 1),
                    )
                nc.scalar.activation(
                    kt[:, lo : lo + 512], ps_k, mybir.ActivationFunctionType.Identity,
                    bias=bk_sb, scale=1.0,
                )
                # V^T quarter -> cast bf16 -> transpose to natural V chunks
                ps_v = pp.tile([P, 512], F32, tag="mm", name=f"psv_{b}_{q4}")
                for cc in range(CK):
                    nc.tensor.matmul(
                        ps_v, lhsT=wv_sb[:, cc], rhs=xq[:, cc],
                        start=(cc == 0), stop=(cc == CK - 1),
                    )
                vt = vp.tile([P, 512], BF16, tag="vt", name=f"vt_{b}_{q4}")
                nc.vector.tensor_copy(vt, ps_v)
                if LEVEL < 4:
                    continue
                # one accumulation group per head: mixing row-base 0/64
                # transposes in one PSUM group faults on hardware
                for h in range(HPC):
                    vtp = pp.tile([P, 4, D], BF16, tag="mm", name=f"vtp_{b}_{q4}_{h}")
                    for t4 in range(4):
                        nc.tensor.matmul(
                            vtp[:, t4],
                            lhsT=vt[h * D : (h + 1) * D, t4 * P : (t4 + 1) * P],
                            rhs=id_sb[h * D : (h + 1) * D, h * D : (h + 1) * D],
                            is_transpose=True,
                            start=(t4 == 0), stop=(t4 == 3),
                        )
                    nc.vector.tensor_copy(vaug[:, q4 * 4 : (q4 + 1) * 4, h, 0:D], vtp)

            # ======== attention per head ========
            attT = qkp.tile([P, T], BF16, tag="attT", name=f"attT_{b}")
            att_j = [
                smallp.tile([P, HPC * D], BF16, tag="attj", bufs=20, name=f"attj_{b}_{j}")
                for j in range(NB)
            ] if LEVEL >= 5 else []
            for h in range(HPC if LEVEL >= 5 else 0):
                hs = h * D
                at_sb = atp.tile([P, AT_W], BF16, tag="at", name=f"at_{b}_{h}")
                # ---- S^T strips + exp + causal mask ----
                for (c, jlo, jhi) in SEGS:
                    w = (jhi - jlo) * P
                    sps = pp.tile([P, 1024], F32, tag="sA", bufs=2, name=f"sps_{b}_{h}_{c}_{jlo}")
                    lhs_k = kt[hs : hs + D, c * P : (c + 1) * P]
                    col = 0
                    while col < w:
                        n = min(512, w - col)
                        nc.tensor.matmul(
                            sps[:, col : col + n],
                            lhsT=lhs_k,
                            rhs=qt[hs : hs + D, jlo * P + col : jlo * P + col + n],
                            start=True, stop=True,
                        )
                        col += n
                    o = AT_OFF[(c, jlo)]
                    nc.scalar.activation(at_sb[:, o : o + w], sps[:, :w], EXP)
                    if jlo == c:  # diagonal block: multiplicative causal mask
                        nc.gpsimd.tensor_tensor(
                            at_sb[:, o : o + P], at_sb[:, o : o + P], mask_sb, MULT
                        )
                # ---- AV phase: att[q, 0:64] + denominator col 64 ----
                for j in range(NB if LEVEL >= 6 else 0):
                    aps = pp.tile([P, D + 1], F32, tag="mm", name=f"aps_{b}_{h}_{j}")
                    for c in range(j + 1):
                        o = AT_OFF[(c, j)]
                        nc.tensor.matmul(
                            aps,
                            lhsT=at_sb[:, o : o + P],
                            rhs=vaug[:, c, h],
                            start=(c == 0), stop=(c == j),
                        )
                    r = smallp.tile([P, 1], F32, tag="r", name=f"r_{b}_{h}_{j}")
                    nc.vector.reciprocal(r, aps[:, D : D + 1])
                    nc.vector.tensor_scalar(
                        att_j[j][:, hs : hs + D], aps[:, 0:D], r, None, MULT
                    )

            # one [128,128] transpose per q-block covers both heads
            for j in range(NB if LEVEL >= 5 else 0):
                tps = pp.tile([P, P], BF16, tag="mm", name=f"tps_{b}_{j}")
                nc.tensor.matmul(
                    tps, lhsT=att_j[j], rhs=id_sb,
                    is_transpose=True, start=True, stop=True,
                )
                nc.vector.tensor_copy(attT[:, j * P : (j + 1) * P], tps)

            # ======== partial projection ========
            if LEVEL < 7:
                continue
            for j in range(NB):
                outst = outp.tile([P, C], F32, tag="outst", name=f"outst_{b}_{j}")
                for n2 in range(2):
                    pps = pp.tile([P, 512], F32, tag="mm", name=f"pps_{b}_{j}_{n2}")
                    nc.tensor.matmul(
                        pps,
                        lhsT=attT[:, j * P : (j + 1) * P],
                        rhs=wp_bf[:, n2 * 512 : (n2 + 1) * 512],
                        start=True, stop=True,
                    )
                    if n2 == 0:
                        nc.vector.tensor_copy(outst[:, n2 * 512 : (n2 + 1) * 512], pps)
                    else:
                        nc.scalar.copy(outst[:, n2 * 512 : (n2 + 1) * 512], pps)
                nc.sync.dma_start(out[b * T + j * P : b * T + (j + 1) * P, :], outst)
        if LEVEL < 7:
            z = outp.tile([P, C], F32, tag="outst", name="zfill")
            nc.vector.memset(z, 0.0)
            nc.sync.dma_start(out[0:P, :], z)


def build_nc():
    nc = bacc.Bacc("TRN2", debug=False, enable_asserts=False, num_devices=NCORES)
    ins = {
        "xt": nc.dram_tensor("xt", [C, B * T], BF16, kind="ExternalInput").ap(),
        "wq": nc.dram_tensor("wq", [C, P], BF16, kind="ExternalInput").ap(),
        "wk": nc.dram_tensor("wk", [C, P], BF16, kind="ExternalInput").ap(),
        "wv": nc.dram_tensor("wv", [C, P], BF16, kind="ExternalInput").ap(),
        "wp": nc.dram_tensor("wp", [P, C], BF16, kind="ExternalInput").ap(),
        "bq": nc.dram_tensor("bq", [P, 1], F32, kind="ExternalInput").ap(),
        "bk": nc.dram_tensor("bk", [P, 1], F32, kind="ExternalInput").ap(),
        "maskt": nc.dram_tensor("maskt", [P, P], BF16, kind="ExternalInput").ap(),
        "ident": nc.dram_tensor("ident", [P, P], BF16, kind="ExternalInput").ap(),
    }
    outs = {"out": nc.dram_tensor("out", [B * T, C], F32, kind="ExternalOutput").ap()}
    with tile.TileContext(nc) as tc:
        attention_body(tc, outs, ins)
    nc.compile()
    return nc


def make_in_maps(inputs, W_qkv, b_qkv, W_proj):
    x2 = np.asarray(inputs, np.float32).reshape(B * T, C)
    xtv = np.ascontiguousarray(x2.T).astype(ml_dtypes.bfloat16)
    W_qkv = np.asarray(W_qkv, np.float32)
    b_qkv = np.asarray(b_qkv, np.float32)
    W_proj = np.asarray(W_proj, np.float32)
    identv = np.eye(P, dtype=ml_dtypes.bfloat16)
    masktv = np.triu(np.ones((P, P), np.float32)).astype(ml_dtypes.bfloat16)
    in_maps = []
    for cid in range(NCORES):
        s = cid * HPC * D
        in_maps.append({
            "xt": xtv,
            "wq": np.ascontiguousarray(W_qkv[:, s : s + P]).astype(ml_dtypes.bfloat16),
            "wk": np.ascontiguousarray(W_qkv[:, C + s : C + s + P]).astype(ml_dtypes.bfloat16),
            "wv": np.ascontiguousarray(W_qkv[:, 2 * C + s : 2 * C + s + P]).astype(ml_dtypes.bfloat16),
            "wp": np.ascontiguousarray(W_proj[s : s + P, :]).astype(ml_dtypes.bfloat16),
            "bq": np.ascontiguousarray(b_qkv[s : s + P].reshape(P, 1) * 0.125),
            "bk": np.ascontiguousarray(b_qkv[C + s : C + s + P].reshape(P, 1)),
            "maskt": masktv,
            "ident": identv,
        })
    return in_maps


_NC_CACHE = {}


def run(inputs, W_qkv, b_qkv, W_proj, b_proj, trace=False, **kw):
    from concourse.bass_utils import run_bass_kernel_spmd

    if "nc" not in _NC_CACHE:
        _NC_CACHE["nc"] = build_nc()
    nc = _NC_CACHE["nc"]
    in_maps = make_in_maps(inputs, W_qkv, b_qkv, W_proj)
    res = run_bass_kernel_spmd(nc, in_maps, core_ids=list(range(NCORES)), trace=trace, **kw)
    acc = res.results[0]["out"].copy()
    for cid in range(1, NCORES):
        acc += res.results[cid]["out"]
    host_bias = np.asarray(b_qkv, np.float32)[2 * C :] @ np.asarray(W_proj, np.float32)
    host_bias = host_bias + np.asarray(b_proj, np.float32)
    outv = (acc + host_bias[None, :]).reshape(B, T, C).astype(np.float32)
    return outv, res


def kernel(inputs, W_qkv, b_qkv, W_proj, b_proj):
    outv, _ = run(inputs, W_qkv, b_qkv, W_proj, b_proj, trace=False)
    return outv



# revision 7
# speedup vs baseline: 1.3658x; 1.3658x over previous
"""Causal self-attention (B=4, T=2048, C=1024, H=16, D=64) on 8 trn2 NeuronCores.

Sharding: tensor-parallel over heads. Each core owns 2 heads:
  - computes Q^T/K^T (d-major) and natural V (via DMA transpose) for its heads
    from the (host-prepacked) full x^T,
  - causal attention (transposed-S flash-style, softmax denominator via an
    augmented ones-column on V),
  - partial output projection with its 128 rows of W_proj, DMA'd straight
    from PSUM.
Host sums the 8 partial projections and adds (b_v @ W_proj + b_proj).

The per-core program is identical (SPMD); only the weight-slice inputs differ.

Schedule: 3-stage software pipeline. Step k emits QKV for batch k interleaved
(instruction-level, cost-weighted) with attention+projection for batch k-1,
so the PE never drains (stays at full p-state) and the scalar engine (exp-only)
runs concurrently with QKV matmuls.

Engine assignment:
  PE:     all matmuls (QKV, S^T strips, AV, proj)
  Scalar: exp only
  Vector: PSUM->SBUF copies (qt/kt bias-add, V^T cast+transp, proj out), recip + normalize
  GpSimd: causal diag masks, ones-column memsets (GpSimd cannot access PSUM)
  Sync:   all DMA issue (loads, DMA-transposes, PSUM->DRAM stores)
"""

import numpy as np
import ml_dtypes

import concourse.bass as bass
import concourse.bacc as bacc
import concourse.mybir as mybir
import concourse.tile as tile

B, T, C, H, D = 4, 2048, 1024, 16, 64
NCORES = 8
HPC = H // NCORES  # heads per core = 2
P = 128
NB = T // P  # 16 blocks of 128 per sequence
CK = C // P  # 8 contraction chunks for the projections

F32 = mybir.dt.float32
BF16 = mybir.dt.bfloat16
ADD = mybir.AluOpType.add
MULT = mybir.AluOpType.mult
EXP = mybir.ActivationFunctionType.Exp


def _segments():
    """(c, jlo, jhi): S^T strip for key-chunk c covering q-blocks [jlo, jhi).
    Width capped at 8 blocks (1024 cols = 2 PSUM banks)."""
    segs = []
    for c in range(NB):
        segs.append((c, c, min(c + 8, NB)))
    for c in range(8):
        if c + 8 < NB:
            segs.append((c, c + 8, NB))
    return segs


SEGS = _segments()


def _at_offsets():
    off = {}
    cur = 0
    for (c, jlo, jhi) in SEGS:
        for j in range(jlo, jhi):
            off[(c, j)] = cur + (j - jlo) * P
        cur += (jhi - jlo) * P
    return off, cur


AT_OFF, AT_W = _at_offsets()  # AT_W = 136*128 = 17408


def attention_body(tc, outs, ins):
    nc = tc.nc
    xt = ins["xt"]          # [P, CK, B*T] bf16 (x^T prepacked p-major)
    wq = ins["wq"]          # [P, CK, P] bf16 (prescaled by 1/sqrt(D))
    wk = ins["wk"]          # [P, CK, P] bf16
    wv = ins["wv"]          # [P, CK, P] bf16
    wp = ins["wp"]          # [P, C] bf16
    bq = ins["bq"]          # [P, 1] f32 (prescaled)
    bk = ins["bk"]          # [P, 1] f32
    maskt = ins["maskt"]    # [P, P] bf16: 1 if k<=q else 0 (S^T layout)
    ident = ins["ident"]    # [P, P] bf16 identity
    out = outs["out"]       # [B*T, C] bf16 partial projection output

    with (
        tc.tile_pool(name="consts", bufs=1) as consts,
        tc.tile_pool(name="xtp", bufs=3) as xtp,
        tc.tile_pool(name="qkp", bufs=2) as qkp,
        tc.tile_pool(name="vtp", bufs=2) as vtp,
        tc.tile_pool(name="vaugp", bufs=2) as vaugp,
        tc.tile_pool(name="atp", bufs=2) as atp,
        tc.tile_pool(name="attjp", bufs=18) as attjp,
        tc.tile_pool(name="attTp", bufs=2) as attTp,
        tc.tile_pool(name="smallp", bufs=6) as smallp,
        tc.tile_pool(name="outp", bufs=3) as outp,
        tc.tile_pool(name="pp", bufs=2, space="PSUM") as pp,
    ):
        # ---- constants ----
        wq_sb = consts.tile([P, CK, P], BF16, name="wq_sb")
        nc.sync.dma_start(wq_sb, wq)
        wk_sb = consts.tile([P, CK, P], BF16, name="wk_sb")
        nc.sync.dma_start(wk_sb, wk)
        wv_sb = consts.tile([P, CK, P], BF16, name="wv_sb")
        nc.sync.dma_start(wv_sb, wv)
        wp_bf = consts.tile([P, C], BF16, name="wp_bf")
        nc.sync.dma_start(wp_bf, wp)
        bq_sb = consts.tile([P, 1], F32, name="bq_sb")
        nc.gpsimd.dma_start(bq_sb, bq)
        bk_sb = consts.tile([P, 1], F32, name="bk_sb")
        nc.gpsimd.dma_start(bk_sb, bk)
        mask_sb = consts.tile([P, P], BF16, name="mask_sb")
        nc.gpsimd.dma_start(mask_sb, maskt)
        id_sb = consts.tile([P, P], BF16, name="id_sb")
        nc.gpsimd.dma_start(id_sb, ident)

        state = {}

        # ---------- unit builders; each unit = (est PE cycles, emit_fn) ----
        def build_qkv_units(b):
            st = {}
            st["qt"] = qkp.tile([P, T], BF16, tag="qt", name=f"qt_{b}")
            st["kt"] = qkp.tile([P, T], BF16, tag="kt", name=f"kt_{b}")
            st["vaug"] = vaugp.tile(
                [P, NB, HPC, D + 1], BF16, tag="vaug", name=f"vaug_{b}"
            )
            st["attj"] = [
                attjp.tile([P, P], BF16, tag="attj", name=f"attj_{b}_{j}")
                for j in range(NB)
            ]
            st["attT"] = attTp.tile([P, T], BF16, tag="attT", name=f"attT_{b}")
            state[b] = st
            qt, kt, vaug = st["qt"], st["kt"], st["vaug"]

            units = []

            def u_ones():
                nc.gpsimd.memset(vaug[:, :, :, D : D + 1], 1.0)

            units.append((0, u_ones))

            def mk_load(q4):
                def fn():
                    lo = b * T + q4 * 512
                    xq = xtp.tile([P, CK, 512], BF16, tag="xq", name=f"xq_{b}_{q4}")
                    nc.sync.dma_start(xq, xt[:, :, lo : lo + 512])
                    st[("xq", q4)] = xq

                return fn

            def mk_q(q4):
                def fn():
                    lo = q4 * 512
                    xq = st[("xq", q4)]
                    ps = pp.tile([P, 512], F32, tag="qkv", name=f"psq_{b}_{q4}")
                    for cc in range(CK):
                        nc.tensor.matmul(
                            ps, lhsT=wq_sb[:, cc], rhs=xq[:, cc],
                            start=(cc == 0), stop=(cc == CK - 1),
                        )
                    nc.vector.tensor_scalar(qt[:, lo : lo + 512], ps, bq_sb, None, ADD)

                return fn

            def mk_k(q4):
                def fn():
                    lo = q4 * 512
                    xq = st[("xq", q4)]
                    ps = pp.tile([P, 512], F32, tag="qkv", name=f"psk_{b}_{q4}")
                    for cc in range(CK):
                        nc.tensor.matmul(
                            ps, lhsT=wk_sb[:, cc], rhs=xq[:, cc],
                            start=(cc == 0), stop=(cc == CK - 1),
                        )
                    nc.vector.tensor_scalar(kt[:, lo : lo + 512], ps, bk_sb, None, ADD)

                return fn

            def mk_v(q4):
                def fn():
                    xq = st[("xq", q4)]
                    ps = pp.tile([P, 512], F32, tag="qkv", name=f"psv_{b}_{q4}")
                    for cc in range(CK):
                        nc.tensor.matmul(
                            ps, lhsT=wv_sb[:, cc], rhs=xq[:, cc],
                            start=(cc == 0), stop=(cc == CK - 1),
                        )
                    vt = vtp.tile([P, 512], BF16, tag="vt", name=f"vt_{b}_{q4}")
                    nc.vector.tensor_copy(vt, ps)
                    st[("vt", q4)] = vt

                return fn

            def mk_vtr(q4, h):
                def fn():
                    vt = st[("vt", q4)]
                    vtr = pp.tile([P, 4, D], BF16, tag="avproj", name=f"vtr_{b}_{q4}_{h}")
                    for t4 in range(4):
                        nc.tensor.matmul(
                            vtr[:, t4],
                            lhsT=vt[h * D : (h + 1) * D, t4 * P : (t4 + 1) * P],
                            rhs=id_sb[h * D : (h + 1) * D, h * D : (h + 1) * D],
                            is_transpose=True,
                            start=(t4 == 0), stop=(t4 == 3),
                        )
                    nc.vector.tensor_copy(vaug[:, q4 * 4 : (q4 + 1) * 4, h, 0:D], vtr)

                return fn

            for q4 in range(4):
                units.append((0, mk_load(q4)))
                units.append((4096, mk_q(q4)))
                units.append((4096, mk_k(q4)))
                units.append((4096, mk_v(q4)))
                units.append((256, mk_vtr(q4, 0)))
                units.append((256, mk_vtr(q4, 1)))
            return units

        def build_att_units(b):
            st = state[b]
            qt, kt, vaug = st["qt"], st["kt"], st["vaug"]
            attj, attT = st["attj"], st["attT"]

            units = []

            def mk_strip(h, c, jlo, jhi):
                def fn():
                    hs = h * D
                    w = (jhi - jlo) * P
                    at_sb = st[("at", h)]
                    sps = pp.tile([P, 1024], F32, tag="sA", name=f"sps_{b}_{h}_{c}_{jlo}")
                    lhs_k = kt[hs : hs + D, c * P : (c + 1) * P]
                    col = 0
                    while col < w:
                        n = min(512, w - col)
                        nc.tensor.matmul(
                            sps[:, col : col + n],
                            lhsT=lhs_k,
                            rhs=qt[hs : hs + D, jlo * P + col : jlo * P + col + n],
                            start=True, stop=True,
                        )
                        col += n
                    o = AT_OFF[(c, jlo)]
                    nc.scalar.activation(at_sb[:, o : o + w], sps[:, :w], EXP)
                    if jlo == c:  # diagonal block: multiplicative causal mask
                        nc.gpsimd.tensor_tensor(
                            at_sb[:, o : o + P], at_sb[:, o : o + P], mask_sb, MULT
                        )

                return fn

            def mk_at_alloc(h):
                def fn():
                    st[("at", h)] = atp.tile(
                        [P, AT_W], BF16, tag="at", name=f"at_{b}_{h}"
                    )

                return fn

            def mk_av(h, j):
                def fn():
                    hs = h * D
                    at_sb = st[("at", h)]
                    aps = pp.tile([P, D + 1], F32, tag="avproj", name=f"aps_{b}_{h}_{j}")
                    for c in range(j + 1):
                        o = AT_OFF[(c, j)]
                        nc.tensor.matmul(
                            aps,
                            lhsT=at_sb[:, o : o + P],
                            rhs=vaug[:, c, h],
                            start=(c == 0), stop=(c == j),
                        )
                    r = smallp.tile([P, 1], F32, tag="r", name=f"r_{b}_{h}_{j}")
                    nc.vector.reciprocal(r, aps[:, D : D + 1])
                    nc.vector.tensor_scalar(
                        attj[j][:, hs : hs + D], aps[:, 0:D], r, None, MULT
                    )

                return fn

            def mk_trans(j):
                def fn():
                    tps = pp.tile([P, P], BF16, tag="avproj", name=f"tps_{b}_{j}")
                    nc.tensor.matmul(
                        tps, lhsT=attj[j], rhs=id_sb,
                        is_transpose=True, start=True, stop=True,
                    )
                    nc.vector.tensor_copy(attT[:, j * P : (j + 1) * P], tps)

                return fn

            def mk_proj(j):
                def fn():
                    outst = outp.tile([P, C], BF16, tag="outst", name=f"outst_{b}_{j}")
                    for n2 in range(2):
                        pps = pp.tile(
                            [P, 512], F32, tag="avproj", name=f"pps_{b}_{j}_{n2}"
                        )
                        nc.tensor.matmul(
                            pps,
                            lhsT=attT[:, j * P : (j + 1) * P],
                            rhs=wp_bf[:, n2 * 512 : (n2 + 1) * 512],
                            start=True, stop=True,
                        )
                        nc.vector.tensor_copy(outst[:, n2 * 512 : (n2 + 1) * 512], pps)
                    nc.sync.dma_start(
                        out[b * T + j * P : b * T + (j + 1) * P, :], outst
                    )

                return fn

            for h in range(HPC):
                units.append((0, mk_at_alloc(h)))
                for (c, jlo, jhi) in SEGS:
                    units.append(((jhi - jlo) * P, mk_strip(h, c, jlo, jhi)))
                for j in range(NB):
                    units.append(((j + 1) * (D + 1), mk_av(h, j)))
                    if h == HPC - 1:
                        units.append((128, mk_trans(j)))
                        if j >= 2:
                            units.append((1024, mk_proj(j - 2)))
            units.append((1024, mk_proj(NB - 2)))
            units.append((1024, mk_proj(NB - 1)))
            return units

        # ---------- merged emission: keep both streams proportionally fed ---
        for k in range(B + 1):
            ua = build_qkv_units(k) if k < B else []
            ub = build_att_units(k - 1) if k >= 1 else []
            ca = sum(c for c, _ in ua) or 1
            cb = sum(c for c, _ in ub) or 1
            ia = ib = 0
            pa = pb = 0
            while ia < len(ua) or ib < len(ub):
                take_a = ib >= len(ub) or (
                    ia < len(ua) and pa * cb < pb * ca
                )
                if take_a:
                    cost, fn = ua[ia]
                    fn()
                    pa += cost
                    ia += 1
                else:
                    cost, fn = ub[ib]
                    fn()
                    pb += cost
                    ib += 1


def build_nc():
    nc = bacc.Bacc("TRN2", debug=False, enable_asserts=False, num_devices=NCORES)
    ins = {
        "xt": nc.dram_tensor("xt", [P, CK, B * T], BF16, kind="ExternalInput").ap(),
        "wq": nc.dram_tensor("wq", [P, CK, P], BF16, kind="ExternalInput").ap(),
        "wk": nc.dram_tensor("wk", [P, CK, P], BF16, kind="ExternalInput").ap(),
        "wv": nc.dram_tensor("wv", [P, CK, P], BF16, kind="ExternalInput").ap(),
        "wp": nc.dram_tensor("wp", [P, C], BF16, kind="ExternalInput").ap(),
        "bq": nc.dram_tensor("bq", [P, 1], F32, kind="ExternalInput").ap(),
        "bk": nc.dram_tensor("bk", [P, 1], F32, kind="ExternalInput").ap(),
        "maskt": nc.dram_tensor("maskt", [P, P], BF16, kind="ExternalInput").ap(),
        "ident": nc.dram_tensor("ident", [P, P], BF16, kind="ExternalInput").ap(),
    }
    outs = {"out": nc.dram_tensor("out", [B * T, C], BF16, kind="ExternalOutput").ap()}
    with tile.TileContext(nc) as tc:
        attention_body(tc, outs, ins)
    nc.compile()
    return nc


def _pack_po(w):  # [C, 128] -> [128, CK, 128] p-major
    return np.ascontiguousarray(
        w.reshape(CK, P, -1).transpose(1, 0, 2)
    )


def make_in_maps(inputs, W_qkv, b_qkv, W_proj):
    x2 = np.asarray(inputs, np.float32).reshape(B * T, C)
    xtv = np.ascontiguousarray(x2.T).astype(ml_dtypes.bfloat16)  # [C, BT]
    xtp = _pack_po(xtv)  # [128, CK, BT]
    W_qkv = np.asarray(W_qkv, np.float32)
    b_qkv = np.asarray(b_qkv, np.float32)
    W_proj = np.asarray(W_proj, np.float32)
    masktv = np.triu(np.ones((P, P), np.float32)).astype(ml_dtypes.bfloat16)
    identv = np.eye(P, dtype=ml_dtypes.bfloat16)
    in_maps = []
    for cid in range(NCORES):
        s = cid * HPC * D
        wq = (W_qkv[:, s : s + P] * 0.125).astype(ml_dtypes.bfloat16)
        wk = W_qkv[:, C + s : C + s + P].astype(ml_dtypes.bfloat16)
        wv = W_qkv[:, 2 * C + s : 2 * C + s + P].astype(ml_dtypes.bfloat16)
        in_maps.append({
            "xt": xtp,
            "wq": _pack_po(wq),
            "wk": _pack_po(wk),
            "wv": _pack_po(wv),
            "wp": np.ascontiguousarray(W_proj[s : s + P, :]).astype(ml_dtypes.bfloat16),
            "bq": np.ascontiguousarray(b_qkv[s : s + P].reshape(P, 1) * 0.125),
            "bk": np.ascontiguousarray(b_qkv[C + s : C + s + P].reshape(P, 1)),
            "maskt": masktv,
            "ident": identv,
        })
    return in_maps


_NC_CACHE = {}


def run(inputs, W_qkv, b_qkv, W_proj, b_proj, trace=False, **kw):
    from concourse.bass_utils import run_bass_kernel_spmd

    if "nc" not in _NC_CACHE:
        _NC_CACHE["nc"] = build_nc()
    nc = _NC_CACHE["nc"]
    in_maps = make_in_maps(inputs, W_qkv, b_qkv, W_proj)
    res = run_bass_kernel_spmd(nc, in_maps, core_ids=list(range(NCORES)), trace=trace, **kw)
    acc = np.zeros((B * T, C), np.float32)
    for cid in range(NCORES):
        acc += np.asarray(res.results[cid]["out"], np.float32)
    host_bias = np.asarray(b_qkv, np.float32)[2 * C :] @ np.asarray(W_proj, np.float32)
    host_bias = host_bias + np.asarray(b_proj, np.float32)
    outv = (acc + host_bias[None, :]).reshape(B, T, C).astype(np.float32)
    return outv, res


def kernel(inputs, W_qkv, b_qkv, W_proj, b_proj):
    outv, _ = run(inputs, W_qkv, b_qkv, W_proj, b_proj, trace=False)
    return outv
